# revision 2
# baseline (speedup 1.0000x reference)
"""Trainium2 Bass kernel for Bahdanau additive cross-attention + softmax +
weighted sum + residual + LayerNorm.

Reference math (per batch element b, all fp32):
    scores[i,j] = sum_d scale[d] * tanh(x[i,d] + context[j,d])     [TQ,TV]
    w = softmax(scores, axis=-1)
    attn[i,d] = sum_j w[i,j] * context[j,d]
    y = x + attn
    out = gamma * (y - mean_d(y)) * rsqrt(var_d(y) + 1e-3) + beta

Sharding: data-parallel over batch B=8, one batch element per NeuronCore.

Per-core strategy (ACT-engine bound — 16.7M tanh evals/core):
  - d lives on the 128 SBUF partitions for the tanh phase.
  - DVE tensor_scalar_add builds S[d, j] = cT[d, j] + x[i, d] (per query i,
    2x perf mode fp32), batched GB query rows per tile.
  - One big ACT Tanh per group ((N+224)/1.2ns amortizes the instruction
    overhead), output cast to fp16.
  - Scores reduction over d via PE matmul with a shifted one-hot scale
    matrix: lhsT = G[:, 128-i : 256-i] where G[:,128] = scale, so row i of
    the PSUM accumulator receives scale . tanh_tile and every other row +=0.
    fp16 operands -> 1 cycle/row on PE (fp32 would be 4x slower).
  - Softmax: DVE reduce_max(negate) -> ACT Exp(bias=-max, accum_out=sum)
    -> DVE reciprocal. Normalization folded into the epilogue.
  - attn^T: PE transpose of w chunks + 4 accumulating fp32 matmuls against
    the natural-layout context tiles.
  - Epilogue: y = attn*r + x (one scalar_tensor_tensor), bn_stats/bn_aggr
    for mean/var, ACT Sqrt(bias=eps) + DVE reciprocal (ACT Rsqrt is banned),
    (y-mu)*rstd in one tensor_scalar, then *gamma +beta. Sqrt deferred to
    the end of both row-tiles so the ACT table set switches only once.
"""

import numpy as np
from contextlib import ExitStack

import concourse.bass as bass
import concourse.bacc as bacc
import concourse.tile as tile
from concourse import mybir
from concourse.masks import make_identity
from concourse.bass_utils import run_bass_kernel_spmd

TQ, TV, D, B = 256, 512, 128, 8
N_CORES = 8
LN_EPS = 1e-3
F32 = mybir.dt.float32

# dtype of the scale-reduction matmul operands (tanh output tile + one-hot
# scale matrix). fp16: 1 cycle/row on PE, ~7e-4 relative quantization.
import os as _os

SCORE_DT = {
    "fp16": mybir.dt.float16,
    "fp32": mybir.dt.float32,
    "fp32r": mybir.dt.float32r,
    "bf16": mybir.dt.bfloat16,
}[_os.environ.get("SCORE_DT", "fp16")]

NG = 16          # tanh groups per 128-row tile
GB = 128 // NG   # query rows per tanh group


def _body(ctx, tc, x_d, c_d, s_d, g_d, b_d, o_d):
    nc = tc.nc
    AF = mybir.ActivationFunctionType
    ALU = mybir.AluOpType

    singles = ctx.enter_context(tc.tile_pool(name="singles", bufs=1))
    s_pool = ctx.enter_context(tc.tile_pool(name="s", bufs=3))
    t_pool = ctx.enter_context(tc.tile_pool(name="t", bufs=3))
    w_pool = ctx.enter_context(tc.tile_pool(name="w", bufs=2))
    vec_pool = ctx.enter_context(tc.tile_pool(name="vec", bufs=4))
    y_pool = ctx.enter_context(tc.tile_pool(name="y", bufs=2))
    out_pool = ctx.enter_context(tc.tile_pool(name="o", bufs=2))
    ps_scores = ctx.enter_context(tc.tile_pool(name="ps_s", bufs=2, space="PSUM"))
    ps_tr = ctx.enter_context(tc.tile_pool(name="ps_t", bufs=2, space="PSUM"))
    ps_attn = ctx.enter_context(tc.tile_pool(name="ps_a", bufs=2, space="PSUM"))

    ident = singles.tile([128, 128], F32)
    make_identity(nc, ident)

    # x rows in natural layout [i(part), t, d] — also the residual input
    xsb = singles.tile([128, 2, D], F32)
    for t in range(2):
        nc.sync.dma_start(xsb[:, t, :], x_d[t * 128:(t + 1) * 128, :])
    # context rows in natural layout [j(part), jc, d] — attn matmul rhs
    csb = singles.tile([128, 4, D], F32)
    for jc in range(4):
        nc.sync.dma_start(csb[:, jc, :], c_d[jc * 128:(jc + 1) * 128, :])

    # transposed copies: xT[d, i], cT[d, j]
    xT = singles.tile([128, TQ], F32)
    for t in range(2):
        pt = ps_tr.tile([128, 128], F32)
        nc.tensor.transpose(pt, xsb[:, t, :], ident)
        nc.vector.tensor_copy(xT[:, t * 128:(t + 1) * 128], pt)
    cT = singles.tile([128, TV], F32)
    for jc in range(4):
        pt = ps_tr.tile([128, 128], F32)
        nc.tensor.transpose(pt, csb[:, jc, :], ident)
        nc.vector.tensor_copy(cT[:, jc * 128:(jc + 1) * 128], pt)

    # shifted one-hot scale matrix: G[:, 128] = scale, 0 elsewhere.
    # lhsT = G[:, 128-i : 256-i] has scale in column i.
    scale_col = singles.tile([128, 1], F32)
    nc.sync.dma_start(scale_col, bass.AP(s_d, 0, [[1, 128], [1, 1]]))
    G = singles.tile([128, 256], SCORE_DT)
    nc.vector.memset(G, 0.0)
    nc.vector.tensor_copy(G[:, 128:129], scale_col)

    # gamma/beta broadcast across partitions
    gamma_b = singles.tile([128, D], F32)
    nc.gpsimd.dma_start(gamma_b, bass.AP(g_d, 0, [[0, 128], [1, 128]]))
    beta_b = singles.tile([128, D], F32)
    nc.gpsimd.dma_start(beta_b, bass.AP(b_d, 0, [[0, 128], [1, 128]]))
    eps_t = singles.tile([128, 1], F32)
    nc.vector.memset(eps_t, LN_EPS)

    saved = []
    for t in range(2):
        scores = ps_scores.tile([128, TV], F32)
        for g in range(NG):
            S = s_pool.tile([128, GB, TV], F32)
            for k in range(GB):
                il = g * GB + k
                iq = t * 128 + il
                nc.vector.tensor_scalar_add(S[:, k, :], cT, xT[:, iq:iq + 1])
            T = t_pool.tile([128, GB, TV], SCORE_DT)
            nc.scalar.activation(T, S, AF.Tanh)
            for k in range(GB):
                il = g * GB + k
                nc.tensor.matmul(
                    scores,
                    G[:, 128 - il:256 - il],
                    T[:, k, :],
                    start=(il == 0),
                    stop=(il == 127),
                )

        neg_max = vec_pool.tile([128, 1], F32)
        nc.vector.reduce_max(neg_max, scores, axis=mybir.AxisListType.X,
                             negate=True)
        w = w_pool.tile([128, TV], F32)
        sum_exp = vec_pool.tile([128, 1], F32)
        nc.scalar.activation(w, scores, AF.Exp, bias=neg_max,
                             accum_out=sum_exp)
        r = vec_pool.tile([128, 1], F32)
        nc.vector.reciprocal(r, sum_exp)

        wT = w_pool.tile([128, 4, 128], F32, tag="wT")
        for jc in range(4):
            pt = ps_tr.tile([128, 128], F32)
            nc.tensor.transpose(pt, w[:, jc * 128:(jc + 1) * 128], ident)
            nc.vector.tensor_copy(wT[:, jc, :], pt)
        attn = ps_attn.tile([128, D], F32)
        for jc in range(4):
            nc.tensor.matmul(attn, wT[:, jc, :], csb[:, jc, :],
                             start=(jc == 0), stop=(jc == 3))

        # y = attn * (1/sum_exp) + x
        y = y_pool.tile([128, D], F32)
        nc.vector.scalar_tensor_tensor(y, in0=attn, scalar=r,
                                       in1=xsb[:, t, :],
                                       op0=ALU.mult, op1=ALU.add)
        stats = vec_pool.tile([128, 6], F32)
        nc.vector.bn_stats(stats, y)
        mv = vec_pool.tile([128, 2], F32)
        nc.vector.bn_aggr(mv, stats)
        saved.append((y, mv))

    # LayerNorm epilogue, deferred so ACT switches to the sqrt table set once
    for t in range(2):
        y, mv = saved[t]
        std = vec_pool.tile([128, 1], F32)
        nc.scalar.activation(std, mv[:, 1:2], AF.Sqrt, bias=eps_t)
        rstd = vec_pool.tile([128, 1], F32)
        nc.vector.reciprocal(rstd, std)
        t1 = out_pool.tile([128, D], F32)
        nc.vector.tensor_scalar(t1, y, mv[:, 0:1], rstd,
                                op0=ALU.subtract, op1=ALU.mult)
        t2 = out_pool.tile([128, D], F32)
        nc.vector.tensor_mul(t2, t1, gamma_b)
        t3 = out_pool.tile([128, D], F32)
        nc.vector.tensor_add(t3, t2, beta_b)
        nc.sync.dma_start(o_d[t * 128:(t + 1) * 128, :], t3)


def build_nc():
    nc = bacc.Bacc("TRN2", target_bir_lowering=False)
    x_d = nc.dram_tensor("x", [TQ, D], F32, kind="ExternalInput")
    c_d = nc.dram_tensor("context", [TV, D], F32, kind="ExternalInput")
    s_d = nc.dram_tensor("scale", [D], F32, kind="ExternalInput")
    g_d = nc.dram_tensor("gamma", [D], F32, kind="ExternalInput")
    b_d = nc.dram_tensor("beta", [D], F32, kind="ExternalInput")
    o_d = nc.dram_tensor("out", [TQ, D], F32, kind="ExternalOutput")
    with tile.TileContext(nc) as tc:
        with ExitStack() as ctx:
            _body(ctx, tc, x_d, c_d, s_d, g_d, b_d, o_d)
    nc.compile()
    return nc


_NC_CACHE = None


def _get_nc():
    global _NC_CACHE
    if _NC_CACHE is None:
        _NC_CACHE = build_nc()
    return _NC_CACHE


def kernel(**inputs) -> np.ndarray:
    x = np.ascontiguousarray(np.asarray(inputs["x"], dtype=np.float32))
    context = np.ascontiguousarray(np.asarray(inputs["context"], dtype=np.float32))
    scale = np.ascontiguousarray(np.asarray(inputs["scale"], dtype=np.float32))
    gamma = np.ascontiguousarray(np.asarray(inputs["gamma"], dtype=np.float32))
    beta = np.ascontiguousarray(np.asarray(inputs["beta"], dtype=np.float32))

    nc = _get_nc()
    in_maps = [
        {
            "x": x[b],
            "context": context[b],
            "scale": scale,
            "gamma": gamma,
            "beta": beta,
        }
        for b in range(B)
    ]
    res = run_bass_kernel_spmd(nc, in_maps, core_ids=list(range(N_CORES)))
    return np.stack([res.results[b]["out"] for b in range(B)], axis=0)


# revision 8
# speedup vs baseline: 297.4714x; 297.4714x over previous
"""Trainium2 Bass kernel for Bahdanau additive cross-attention + softmax +
weighted sum + residual + LayerNorm.

Reference math (per batch element b, all fp32):
    scores[i,j] = sum_d scale[d] * tanh(x[i,d] + context[j,d])     [TQ,TV]
    w = softmax(scores, axis=-1)
    attn[i,d] = sum_j w[i,j] * context[j,d]
    y = x + attn
    out = gamma * (y - mean_d(y)) * rsqrt(var_d(y) + 1e-3) + beta

Sharding: data-parallel over batch B=8, one batch element per NeuronCore.

Per-core strategy (ACT-engine bound — 16.7M tanh evals/core):
  - d lives on the 128 SBUF partitions for the tanh phase.
  - DVE tensor_scalar_add builds S[d, j] = cT[d, j] + x[i, d] (per query i,
    2x perf mode fp32), batched GB query rows per tile.
  - One big ACT Tanh per group ((N+224)/1.2ns amortizes the instruction
    overhead), output cast to fp16.
  - Scores reduction over d via PE matmul with a shifted one-hot scale
    matrix: lhsT = G[:, 128-i : 256-i] where G[:,128] = scale, so row i of
    the PSUM accumulator receives scale . tanh_tile and every other row +=0.
    fp16 operands -> 1 cycle/row on PE (fp32 would be 4x slower).
  - Softmax: DVE reduce_max(negate) -> ACT Exp(bias=-max, accum_out=sum)
    -> DVE reciprocal. Normalization folded into the epilogue.
  - attn^T: PE transpose of w chunks + 4 accumulating fp32 matmuls against
    the natural-layout context tiles.
  - Epilogue: y = attn*r + x (one scalar_tensor_tensor), bn_stats/bn_aggr
    for mean/var, ACT Sqrt(bias=eps) + DVE reciprocal (ACT Rsqrt is banned),
    (y-mu)*rstd in one tensor_scalar, then *gamma +beta. Sqrt deferred to
    the end of both row-tiles so the ACT table set switches only once.
"""

import numpy as np
from contextlib import ExitStack

import concourse.bass as bass
import concourse.bacc as bacc
import concourse.tile as tile
from concourse import mybir
from concourse.masks import make_identity
from concourse.bass_utils import run_bass_kernel_spmd

TQ, TV, D, B = 256, 512, 128, 8
N_CORES = 8
LN_EPS = 1e-3
F32 = mybir.dt.float32

# dtype of the scale-reduction matmul operands (tanh output tile + one-hot
# scale matrix). fp16: 1 cycle/row on PE, ~7e-4 relative quantization.
import os as _os

SCORE_DT = {
    "fp16": mybir.dt.float16,
    "fp32": mybir.dt.float32,
    "fp32r": mybir.dt.float32r,
    "bf16": mybir.dt.bfloat16,
}[_os.environ.get("SCORE_DT", "fp16")]

NG = 16          # tanh groups per 128-row tile
GB = 128 // NG   # query rows per tanh group


def _body(ctx, tc, x_d, c_d, s_d, g_d, b_d, o_d, repeats=1, loop_iters=1):
    nc = tc.nc
    AF = mybir.ActivationFunctionType
    ALU = mybir.AluOpType

    singles = ctx.enter_context(tc.tile_pool(name="singles", bufs=1))
    s_pool = ctx.enter_context(tc.tile_pool(name="s", bufs=3))
    t_pool = ctx.enter_context(tc.tile_pool(name="t", bufs=3))
    w_pool = ctx.enter_context(tc.tile_pool(name="w", bufs=2))
    vec_pool = ctx.enter_context(tc.tile_pool(name="vec", bufs=4))
    y_pool = ctx.enter_context(tc.tile_pool(name="y", bufs=2))
    out_pool = ctx.enter_context(tc.tile_pool(name="o", bufs=2))
    ps_scores = ctx.enter_context(tc.tile_pool(name="ps_s", bufs=2, space="PSUM"))
    ps_tr = ctx.enter_context(tc.tile_pool(name="ps_t", bufs=2, space="PSUM"))
    ps_attn = ctx.enter_context(tc.tile_pool(name="ps_a", bufs=2, space="PSUM"))

    ident = singles.tile([128, 128], F32)
    make_identity(nc, ident)

    # x rows in natural layout [i(part), t, d] — also the residual input
    xsb = singles.tile([128, 2, D], F32)
    for t in range(2):
        nc.sync.dma_start(xsb[:, t, :], x_d[t * 128:(t + 1) * 128, :])
    # context rows in natural layout [j(part), jc, d] — attn matmul rhs
    csb = singles.tile([128, 4, D], F32)
    for jc in range(4):
        nc.sync.dma_start(csb[:, jc, :], c_d[jc * 128:(jc + 1) * 128, :])

    # transposed copies: xT[d, i], cT[d, j]
    xT = singles.tile([128, TQ], F32)
    for t in range(2):
        pt = ps_tr.tile([128, 128], F32)
        nc.tensor.transpose(pt, xsb[:, t, :], ident)
        nc.vector.tensor_copy(xT[:, t * 128:(t + 1) * 128], pt)
    cT = singles.tile([128, TV], F32)
    for jc in range(4):
        pt = ps_tr.tile([128, 128], F32)
        nc.tensor.transpose(pt, csb[:, jc, :], ident)
        nc.vector.tensor_copy(cT[:, jc * 128:(jc + 1) * 128], pt)

    # shifted one-hot scale matrix: G[:, 128] = scale, 0 elsewhere.
    # lhsT = G[:, 128-i : 256-i] has scale in column i.
    scale_col = singles.tile([128, 1], F32)
    nc.sync.dma_start(scale_col, bass.AP(s_d, 0, [[1, 128], [1, 1]]))
    G = singles.tile([128, 256], SCORE_DT)
    nc.vector.memset(G, 0.0)
    nc.vector.tensor_copy(G[:, 128:129], scale_col)

    # gamma/beta broadcast across partitions
    gamma_b = singles.tile([128, D], F32)
    nc.gpsimd.dma_start(gamma_b, bass.AP(g_d, 0, [[0, 128], [1, 128]]))
    beta_b = singles.tile([128, D], F32)
    nc.gpsimd.dma_start(beta_b, bass.AP(b_d, 0, [[0, 128], [1, 128]]))
    eps_t = singles.tile([128, 1], F32)
    nc.vector.memset(eps_t, LN_EPS)

    if loop_iters > 1:
        env = locals()
        with tc.For_i(0, loop_iters, 1):
            _main_pass(tc, ctx, env)
    else:
        for _rep in range(repeats):
            _main_pass(tc, ctx, locals())


def _main_pass(tc, ctx, env):
    nc = tc.nc
    AF = mybir.ActivationFunctionType
    ALU = mybir.AluOpType
    (s_pool, t_pool, w_pool, vec_pool, y_pool, out_pool, ps_scores, ps_tr,
     ps_attn, ident, xsb, csb, xT, cT, G, gamma_b, beta_b, eps_t, o_d) = (
        env["s_pool"], env["t_pool"], env["w_pool"], env["vec_pool"],
        env["y_pool"], env["out_pool"], env["ps_scores"], env["ps_tr"],
        env["ps_attn"], env["ident"], env["xsb"], env["csb"], env["xT"],
        env["cT"], env["G"], env["gamma_b"], env["beta_b"], env["eps_t"],
        env["o_d"])

    saved = []
    for t in range(2):
        scores = ps_scores.tile([128, TV], F32)
        for g in range(NG):
            S = s_pool.tile([128, GB, TV], F32)
            for k in range(GB):
                il = g * GB + k
                iq = t * 128 + il
                nc.vector.tensor_scalar_add(S[:, k, :], cT, xT[:, iq:iq + 1])
            T = t_pool.tile([128, GB, TV], SCORE_DT)
            nc.scalar.activation(T, S, AF.Tanh)
            for k in range(GB):
                il = g * GB + k
                nc.tensor.matmul(
                    scores,
                    G[:, 128 - il:256 - il],
                    T[:, k, :],
                    start=(il == 0),
                    stop=(il == 127),
                )

        neg_max = vec_pool.tile([128, 1], F32)
        nc.vector.reduce_max(neg_max, scores, axis=mybir.AxisListType.X,
                             negate=True)
        w = w_pool.tile([128, TV], F32)
        sum_exp = vec_pool.tile([128, 1], F32)
        nc.scalar.activation(w, scores, AF.Exp, bias=neg_max,
                             accum_out=sum_exp)
        r = vec_pool.tile([128, 1], F32)
        nc.vector.reciprocal(r, sum_exp)

        wT = w_pool.tile([128, 4, 128], F32, tag="wT")
        for jc in range(4):
            pt = ps_tr.tile([128, 128], F32)
            nc.tensor.transpose(pt, w[:, jc * 128:(jc + 1) * 128], ident)
            nc.vector.tensor_copy(wT[:, jc, :], pt)
        attn = ps_attn.tile([128, D], F32)
        for jc in range(4):
            nc.tensor.matmul(attn, wT[:, jc, :], csb[:, jc, :],
                             start=(jc == 0), stop=(jc == 3))

        # y = attn * (1/sum_exp) + x
        y = y_pool.tile([128, D], F32)
        nc.vector.scalar_tensor_tensor(y, in0=attn, scalar=r,
                                       in1=xsb[:, t, :],
                                       op0=ALU.mult, op1=ALU.add)
        stats = vec_pool.tile([128, 6], F32)
        nc.vector.bn_stats(stats, y)
        mv = vec_pool.tile([128, 2], F32)
        nc.vector.bn_aggr(mv, stats)
        saved.append((y, mv))

    # LayerNorm epilogue, deferred so ACT switches to the sqrt table set once
    for t in range(2):
        y, mv = saved[t]
        std = vec_pool.tile([128, 1], F32)
        nc.scalar.activation(std, mv[:, 1:2], AF.Sqrt, bias=eps_t)
        rstd = vec_pool.tile([128, 1], F32)
        nc.vector.reciprocal(rstd, std)
        t1 = out_pool.tile([128, D], F32)
        nc.vector.tensor_scalar(t1, y, mv[:, 0:1], rstd,
                                op0=ALU.subtract, op1=ALU.mult)
        t2 = out_pool.tile([128, D], F32)
        nc.vector.tensor_mul(t2, t1, gamma_b)
        t3 = out_pool.tile([128, D], F32)
        nc.vector.tensor_add(t3, t2, beta_b)
        nc.sync.dma_start(o_d[t * 128:(t + 1) * 128, :], t3)


def build_nc(repeats=1, loop_iters=1):
    nc = bacc.Bacc("TRN2", target_bir_lowering=False)
    x_d = nc.dram_tensor("x", [TQ, D], F32, kind="ExternalInput")
    c_d = nc.dram_tensor("context", [TV, D], F32, kind="ExternalInput")
    s_d = nc.dram_tensor("scale", [D], F32, kind="ExternalInput")
    g_d = nc.dram_tensor("gamma", [D], F32, kind="ExternalInput")
    b_d = nc.dram_tensor("beta", [D], F32, kind="ExternalInput")
    o_d = nc.dram_tensor("out", [TQ, D], F32, kind="ExternalOutput")
    with tile.TileContext(nc) as tc:
        with ExitStack() as ctx:
            _body(ctx, tc, x_d, c_d, s_d, g_d, b_d, o_d, repeats=repeats,
                  loop_iters=loop_iters)
    nc.compile()
    return nc


_NC_CACHE = None


def _get_nc():
    global _NC_CACHE
    if _NC_CACHE is None:
        _NC_CACHE = build_nc()
    return _NC_CACHE


def kernel(**inputs) -> np.ndarray:
    x = np.ascontiguousarray(np.asarray(inputs["x"], dtype=np.float32))
    context = np.ascontiguousarray(np.asarray(inputs["context"], dtype=np.float32))
    scale = np.ascontiguousarray(np.asarray(inputs["scale"], dtype=np.float32))
    gamma = np.ascontiguousarray(np.asarray(inputs["gamma"], dtype=np.float32))
    beta = np.ascontiguousarray(np.asarray(inputs["beta"], dtype=np.float32))

    nc = _get_nc()
    in_maps = [
        {
            "x": x[b],
            "context": context[b],
            "scale": scale,
            "gamma": gamma,
            "beta": beta,
        }
        for b in range(B)
    ]
    res = run_bass_kernel_spmd(nc, in_maps, core_ids=list(range(N_CORES)))
    return np.stack([res.results[b]["out"] for b in range(B)], axis=0)
